# revision 9
# baseline (speedup 1.0000x reference)
"""Multi-head self-attention (RoPE, causal) Trainium2 Bass kernel, 8 NeuronCores.

Sharding: data-parallel over batch (B=2) x tensor-parallel over heads
(16 heads -> 4 groups of 4). Core c handles batch b=c//4, heads 4*(c%4)..4*(c%4)+3.
Each core computes its 4 heads' attention plus a partial output projection;
the host sums the 4 partial outputs per batch element (bf16 partials, f32 sum).

v2 structure: single software pipeline over 512-wide q chunks. Engines run
their instruction queues in order, so projection work for chunk qc+1 and the
output projection for chunk qc-1 are emitted interleaved into chunk qc's
attention kt stream to fill PE gaps while the scalar engine runs exp.
  - QK proj psum -> ACT-evict bf16 -> RoPE on DVE/GpSimd (shuffle/mul/mul/add).
  - Scores per (pair, kt) in [k, q] layout, two row-tiled 64-contraction
    matmuls; exp on ACT (scale folded); causal via skipped tiles + diagonal
    mask-mul on DVE.
  - PV accumulates into one 2-bank psum tile po [0:65, 1024] with the fused
    ones-row denominators at row 64 (both heads).
  - Softmax normalize: gpsimd partition_broadcast of the psum denominator row
    (no DRAM round trip), one DVE reciprocal, bf16 DVE multiplies into at;
    the h1 half is shifted to partitions 64:128 by a small PSUM->SBUF DMA.
PSUM: pj pool 2 banks (proj/V/outproj) + scores 4 + po 2 = 8.
"""
import sys, math

sys.path.insert(0, "/opt/trn_rl_repo")

import numpy as np
import ml_dtypes

import concourse.bacc as bacc
import concourse.bass as bass
import concourse.mybir as mybir
import concourse.tile as tile
from concourse.bass_utils import run_bass_kernel_spmd

BF16 = mybir.dt.bfloat16
F32 = mybir.dt.float32
NPBF16 = ml_dtypes.bfloat16

D_MODEL = 1024
D_HEAD = 64
HALF = D_HEAD // 2
ROPE_THETA = 10000.0
N_CORES = 8
C = 256  # channels per core (4 heads x 64)
SWAP32 = [i ^ 1 for i in range(32)]


def _body(nc, tc, L, pp, rtp, ptp, nrp, osp, scrp):
    n_qc = L // 512

    xt_d = nc.dram_tensor("xt", [D_MODEL, L], BF16, kind="ExternalInput").ap()
    wq_d = nc.dram_tensor("wqt", [D_MODEL, C], BF16, kind="ExternalInput").ap()
    wk_d = nc.dram_tensor("wkt", [D_MODEL, C], BF16, kind="ExternalInput").ap()
    wv_d = nc.dram_tensor("wvt", [D_MODEL, C], BF16, kind="ExternalInput").ap()
    wo_d = nc.dram_tensor("wot", [C, D_MODEL], BF16, kind="ExternalInput").ap()
    cos_d = nc.dram_tensor("cosb", [128, L], BF16, kind="ExternalInput").ap()
    sin_d = nc.dram_tensor("ssin", [128, L], BF16, kind="ExternalInput").ap()
    mk_d = nc.dram_tensor("masks", [128, 128], BF16, kind="ExternalInput").ap()
    out_d = nc.dram_tensor("out", [L, D_MODEL], BF16, kind="ExternalOutput").ap()

    # ---- persistent SBUF tensors
    wq = pp.tile([128, 8, C], BF16)
    wk = pp.tile([128, 8, C], BF16)
    wv = pp.tile([128, 8, C], BF16)
    wo = pp.tile([128, 2, D_MODEL], BF16)
    cs = pp.tile([128, L], BF16)
    sn = pp.tile([128, L], BF16)
    mks = pp.tile([128, 128], BF16)
    qt_c = [pp.tile([128, 2, 512], BF16, name=f"qt{i}") for i in range(n_qc)]
    kt_c = [pp.tile([128, 2, 512], BF16, name=f"ktc{i}") for i in range(n_qc)]
    # per head h: cols 65h..65h+63 = V, col 65h+64 = ones (denominator row)
    vt_c = [pp.tile([128, 4, 4 * 65], BF16, name=f"vt{i}") for i in range(n_qc)]
    at = pp.tile([128, 2, L], BF16)
    xtc = [[pp.tile([128, 512], BF16, name=f"xt{d}_{q}") for q in range(n_qc)]
           for d in range(8)]

    # ---- loads: priority order (first compute needs wq/wk + x chunk 0)
    nc.sync.dma_start(out=wq[:], in_=wq_d.rearrange("(a p) c -> p a c", p=128))
    nc.sync.dma_start(out=wk[:], in_=wk_d.rearrange("(a p) c -> p a c", p=128))
    for d in range(8):
        nc.sync.dma_start(out=xtc[d][0][:],
                          in_=xt_d[d * 128:(d + 1) * 128, 0:512])
    nc.sync.dma_start(out=cs[:], in_=cos_d)
    nc.sync.dma_start(out=sn[:], in_=sin_d)
    nc.sync.dma_start(out=wv[:], in_=wv_d.rearrange("(a p) c -> p a c", p=128))
    nc.sync.dma_start(out=mks[:], in_=mk_d)
    for q in range(1, n_qc):
        for d in range(8):
            nc.sync.dma_start(out=xtc[d][q][:],
                              in_=xt_d[d * 128:(d + 1) * 128,
                                       q * 512:q * 512 + 512])
    nc.sync.dma_start(out=wo[:], in_=wo_d.rearrange("(a p) e -> p a e", p=128))
    for i in range(n_qc):
        ones_v = vt_c[i][:, :, :].rearrange("p l (h y) -> p l h y", y=65)
        nc.gpsimd.memset(ones_v[:, :, :, 64], 1.0)

    # ---------------- emission units (thunk lists for PE interleaving)
    def qk_units(qc):
        """QK projection + RoPE for chunk qc -> 8 units."""
        units = []
        for nm, w, dstc in (("q", wq, qt_c), ("k", wk, kt_c)):
            for ct in (0, 1):
                st = {}

                def mk_a(nm=nm, w=w, ct=ct, qc=qc, st=st):
                    p = ptp.tile([128, 512], F32, tag="pj",
                                 name=f"pj_{nm}{ct}_{qc}")
                    st["p"] = p
                    for dt_ in range(4):
                        nc.tensor.matmul(
                            p[:], lhsT=w[:, dt_, ct * 128:ct * 128 + 128],
                            rhs=xtc[dt_][qc][:],
                            start=(dt_ == 0), stop=False)

                def mk_b(nm=nm, w=w, ct=ct, qc=qc, st=st, dst=None):
                    p = st["p"]
                    for dt_ in range(4, 8):
                        nc.tensor.matmul(
                            p[:], lhsT=w[:, dt_, ct * 128:ct * 128 + 128],
                            rhs=xtc[dt_][qc][:],
                            start=False, stop=(dt_ == 7))
                    qraw = rtp.tile([128, 512], BF16, tag="qraw",
                                    name=f"qraw_{nm}{ct}{qc}")
                    nc.scalar.copy(qraw[:], p[:])
                    sh = rtp.tile([128, 512], BF16, tag="sh",
                                  name=f"sh_{nm}{ct}{qc}")
                    t1 = rtp.tile([128, 512], BF16, tag="t1",
                                  name=f"t1_{nm}{ct}{qc}")
                    t2 = rtp.tile([128, 512], BF16, tag="t2",
                                  name=f"t2_{nm}{ct}{qc}")
                    ls = qc * 512
                    nc.vector.stream_shuffle(sh[:], qraw[:], SWAP32)
                    nc.vector.tensor_mul(t1[:], qraw[:], cs[:, ls:ls + 512])
                    nc.gpsimd.tensor_mul(t2[:], sh[:], sn[:, ls:ls + 512])
                    dstt = (qt_c if nm == "q" else kt_c)[qc]
                    nc.vector.tensor_add(dstt[:, ct, :], t1[:], t2[:])

                units.append(mk_a)
                units.append(mk_b)
        return units

    def v_units(qc):
        """V projection for chunk qc -> 4 units (one per 128-L tile)."""
        units = []
        for j in range(4):
            def mk(qc=qc, j=j):
                pv = ptp.tile([128, 256], F32, tag="pj",
                              name=f"pv_{qc}_{j}")
                lt = qc * 4 + j
                for dt_ in range(8):
                    nc.tensor.matmul(
                        pv[:], lhsT=xtc[dt_][qc][:, j * 128:j * 128 + 128],
                        rhs=wv[:, dt_, :],
                        start=(dt_ == 0), stop=(dt_ == 7))
                ov = vt_c[qc][:, j, :].rearrange("p (h y) -> p h y", y=65)
                iv = pv[:, :].rearrange("p (h y) -> p h y", y=64)
                nc.scalar.copy(ov[:, :, 0:64], iv[:, :, :])
            units.append(mk)
        return units

    def op_units(qc):
        """Output projection for chunk qc -> 8 units."""
        units = []
        for j in range(4):
            for eh in (0, 1):
                def mk(qc=qc, j=j, eh=eh):
                    qtl = qc * 4 + j
                    pout = ptp.tile([128, 512], F32, tag="pj",
                                    name=f"po_{qtl}_{eh}")
                    for ct in (0, 1):
                        nc.tensor.matmul(
                            pout[:],
                            lhsT=at[:, ct, qtl * 128:qtl * 128 + 128],
                            rhs=wo[:, ct, eh * 512:eh * 512 + 512],
                            start=(ct == 0), stop=(ct == 1),
                            skip_group_check=True)
                    stg = osp.tile([128, 512], BF16, tag="stg",
                                   name=f"stg_{qtl}_{eh}")
                    if (j + eh) % 2 == 0:
                        nc.vector.tensor_copy(stg[:], pout[:])
                    else:
                        nc.scalar.copy(stg[:], pout[:])
                    nc.sync.dma_start(
                        out=out_d[qtl * 128:qtl * 128 + 128,
                                  eh * 512:eh * 512 + 512],
                        in_=stg[:])
                units.append(mk)
        return units

    # ---------------- attention chunk with interleaved background units
    def attention_chunk(qc, bg):
        qs = qc * 512
        ktmax = qc * 4 + 4
        n_slots = 2 * ktmax
        # pop roughly len(bg)/n_slots units per kt slot
        popped = 0
        done = 0
        for pair in range(2):
            h0, h1 = 2 * pair, 2 * pair + 1
            po = ptp.tile([128, 1024], F32, tag="po", bufs=1,
                          name=f"poacc_{qc}_{pair}")
            for kt in range(ktmax):
                off = kt * 128 - qs
                qlo = max(0, off)
                kc, ko = kt // 4, (kt % 4) * 128
                pt = ptp.tile([128, 1024], F32, tag="sc",
                              name=f"pt_{qc}_{pair}_{kt}")
                for hloc in range(2):
                    nc.tensor.matmul(
                        pt[:, 512 * hloc + qlo:512 * hloc + 512],
                        lhsT=kt_c[kc][64 * hloc:64 * hloc + 64, pair,
                                      ko:ko + 128],
                        rhs=qt_c[qc][64 * hloc:64 * hloc + 64, pair,
                                     qlo:512],
                        start=True, stop=True,
                        tile_position=(64 * hloc, 0),
                        skip_group_check=True)
                ptb = rtp.tile([128, 1024], BF16, tag="ptb", bufs=3,
                               name=f"ptb_{qc}_{pair}_{kt}")
                pv_ps = pt[:, :].rearrange("p (h x) -> p h x", h=2)
                pv_sb = ptb[:, :].rearrange("p (h x) -> p h x", h=2)
                nc.scalar.activation(pv_sb[:, :, qlo:512], pv_ps[:, :, qlo:512],
                                     mybir.ActivationFunctionType.Exp,
                                     scale=1.0 / math.sqrt(D_HEAD))
                if off >= 0:
                    for hloc in range(2):
                        nc.vector.tensor_mul(
                            ptb[:, 512 * hloc + qlo:512 * hloc + qlo + 128],
                            ptb[:, 512 * hloc + qlo:512 * hloc + qlo + 128],
                            mks[:, 0:128])
                first, last = (kt == 0), (kt == ktmax - 1)
                nc.tensor.matmul(
                    po[0:65, qlo:512],
                    lhsT=vt_c[kc][:, kt % 4, 65 * h0:65 * h0 + 65],
                    rhs=ptb[:, qlo:512],
                    start=first, stop=last, skip_group_check=True)
                nc.tensor.matmul(
                    po[0:65, 512 + qlo:1024],
                    lhsT=vt_c[kc][:, kt % 4, 65 * h1:65 * h1 + 65],
                    rhs=ptb[:, 512 + qlo:1024],
                    start=first, stop=last, skip_group_check=True)
                # interleave background PE work
                done += 1
                want = (len(bg) * done) // n_slots
                while popped < want:
                    bg[popped]()
                    popped += 1
            # ---- finalize pair: denominator row -> DRAM -> partition
            # broadcast -> reciprocal -> normalize on partitions 0:64; h1
            # half shifted up to partitions 64:128 by an SBUF->SBUF DMA.
            atmp = nrp.tile([128, 1024], BF16, tag="atmp",
                            name=f"atmp_{qc}_{pair}")
            nc.scalar.copy(atmp[0:64, :], po[0:64, :])
            rrow = nrp.tile([1, 1024], F32, tag="rrow", name=f"rrow_{qc}_{pair}")
            nc.vector.tensor_copy(rrow[:], po[64:65, :])
            scrt = scrp.tile([1, 1024], F32, tag="scr", name=f"scr_{qc}_{pair}")
            nc.sync.dma_start(out=scrt[:, :], in_=rrow[:])
            pbr = nrp.tile([64, 1024], F32, tag="pbr", name=f"pbr_{qc}_{pair}")
            nc.sync.dma_start(out=pbr[:, :], in_=scrt[:, :].partition_broadcast(64))
            pbi = nrp.tile([64, 1024], F32, tag="pbi", name=f"pbi_{qc}_{pair}")
            nc.vector.reciprocal_approx_fast(out=pbi[:, :], in_=pbr[:, :])
            nc.vector.tensor_mul(at[0:64, pair, qs:qs + 512],
                                 atmp[0:64, 0:512], pbi[:, 0:512])
            th1 = nrp.tile([64, 512], BF16, tag="th1", name=f"th1_{qc}_{pair}")
            nc.vector.tensor_mul(th1[:, :], atmp[0:64, 512:1024],
                                 pbi[:, 512:1024])
            nc.sync.dma_start(out=at[64:128, pair, qs:qs + 512], in_=th1[:, :])
        while popped < len(bg):
            bg[popped]()
            popped += 1

    # ---------------- pipeline
    for u in qk_units(0) + v_units(0):
        u()
    for qc in range(n_qc):
        bg = []
        if qc + 1 < n_qc:
            bg += qk_units(qc + 1) + v_units(qc + 1)
        if qc > 0:
            bg += op_units(qc - 1)
        attention_chunk(qc, bg)
    for u in op_units(n_qc - 1):
        u()


def build_nc(L=2048):
    """Build + compile the per-core Bass program (same NEFF on all 8 cores)."""
    assert L % 512 == 0
    nc = bacc.Bacc("TRN2", target_bir_lowering=False, debug=False,
                   num_devices=N_CORES)
    with tile.TileContext(nc) as tc:
        with tc.tile_pool(name="persist", bufs=1) as pp, \
             tc.tile_pool(name="ropet", bufs=3) as rtp, \
             tc.tile_pool(name="psmix", bufs=2, space="PSUM") as ptp, \
             tc.tile_pool(name="norm", bufs=2) as nrp, \
             tc.tile_pool(name="ostg", bufs=3) as osp, \
             tc.tile_pool(name="riscr", bufs=4, space="DRAM") as scrp:
            _body(nc, tc, L, pp, rtp, ptp, nrp, osp, scrp)
    nc.compile()
    return nc


_NC_CACHE = {}


def _get_nc(L):
    if L not in _NC_CACHE:
        _NC_CACHE[L] = build_nc(L)
    return _NC_CACHE[L]


def make_inputs(x, token_positions, Wq, Wk, Wv, Wo):
    """Host-side shard/layout prep -> list of 8 per-core input dicts."""
    B, L, _ = x.shape
    pos = np.asarray(token_positions).astype(np.float64)
    S = ROPE_THETA ** (-2.0 / D_HEAD)
    thetas = S ** np.arange(HALF, dtype=np.float64)
    ang = pos[:, None] * thetas[None, :]          # [L, 32]
    cosL = np.cos(ang).T                          # [32, L]
    sinL = np.sin(ang).T
    # per-channel tables on the natural (head, dim) layout:
    # row p (within a 64-row head block): pair i = (p%64)//2
    # cosb[p] = cos(theta_i * pos); ssin[p] = -sin if dim even else +sin
    cosb = np.empty((128, L), dtype=np.float64)
    ssin = np.empty((128, L), dtype=np.float64)
    for p in range(128):
        i = (p % 64) // 2
        cosb[p] = cosL[i]
        ssin[p] = -sinL[i] if (p % 2 == 0) else sinL[i]
    cosb = cosb.astype(NPBF16)
    ssin = ssin.astype(NPBF16)

    r = np.arange(128)[:, None]
    col = np.arange(128)[None, :]
    masks = (col >= r).astype(NPBF16)  # [128, 128] tril(keep q>=k)

    xts = [np.ascontiguousarray(x[b].astype(NPBF16).T) for b in range(B)]
    in_maps = []
    shard_cache = {}
    for core in range(N_CORES):
        b, hg = core // 4, core % 4
        if hg not in shard_cache:
            rows = slice(hg * 256, hg * 256 + 256)
            shard_cache[hg] = {
                "wqt": np.ascontiguousarray(Wq[rows].astype(NPBF16).T),
                "wkt": np.ascontiguousarray(Wk[rows].astype(NPBF16).T),
                "wvt": np.ascontiguousarray(Wv[rows].astype(NPBF16).T),
                "wot": np.ascontiguousarray(Wo[:, rows].astype(NPBF16).T),
            }
        m = dict(shard_cache[hg])
        m["xt"] = xts[b]
        m["cosb"] = cosb
        m["ssin"] = ssin
        m["masks"] = masks
        in_maps.append(m)
    return in_maps


def kernel(x, token_positions, Wq, Wk, Wv, Wo):
    x = np.asarray(x); Wq = np.asarray(Wq); Wk = np.asarray(Wk)
    Wv = np.asarray(Wv); Wo = np.asarray(Wo)
    B, L, _ = x.shape
    nc = _get_nc(L)
    in_maps = make_inputs(x, token_positions, Wq, Wk, Wv, Wo)
    res = run_bass_kernel_spmd(nc, in_maps, core_ids=list(range(N_CORES)))
    out = np.zeros((B, L, D_MODEL), dtype=np.float32)
    for core in range(N_CORES):
        out[core // 4] += res.results[core]["out"].astype(np.float32)
    return out


# revision 11
# speedup vs baseline: 1.0632x; 1.0632x over previous
"""Multi-head self-attention (RoPE, causal) Trainium2 Bass kernel, 8 NeuronCores.

Sharding: data-parallel over batch (B=2) x tensor-parallel over heads
(16 heads -> 4 groups of 4). Core c handles batch b=c//4, heads 4*(c%4)..4*(c%4)+3.
Each core computes its 4 heads' attention plus a partial output projection;
the host sums the 4 partial outputs per batch element (bf16 partials, f32 sum).

v2 structure: single software pipeline over 512-wide q chunks. Engines run
their instruction queues in order, so projection work for chunk qc+1 and the
output projection for chunk qc-1 are emitted interleaved into chunk qc's
attention kt stream to fill PE gaps while the scalar engine runs exp.
  - QK proj psum -> ACT-evict bf16 -> RoPE on DVE/GpSimd (shuffle/mul/mul/add).
  - Scores per (pair, kt) in [k, q] layout, two row-tiled 64-contraction
    matmuls; exp on ACT (scale folded); causal via skipped tiles + diagonal
    mask-mul on DVE.
  - PV accumulates into one 2-bank psum tile po [0:65, 1024] with the fused
    ones-row denominators at row 64 (both heads).
  - Softmax normalize: gpsimd partition_broadcast of the psum denominator row
    (no DRAM round trip), one DVE reciprocal, bf16 DVE multiplies into at;
    the h1 half is shifted to partitions 64:128 by a small PSUM->SBUF DMA.
PSUM: pj pool 2 banks (proj/V/outproj) + scores 4 + po 2 = 8.
"""
import sys, math

sys.path.insert(0, "/opt/trn_rl_repo")

import numpy as np
import ml_dtypes

import concourse.bacc as bacc
import concourse.bass as bass
import concourse.mybir as mybir
import concourse.tile as tile
from concourse.bass_utils import run_bass_kernel_spmd

BF16 = mybir.dt.bfloat16
F32 = mybir.dt.float32
NPBF16 = ml_dtypes.bfloat16

D_MODEL = 1024
D_HEAD = 64
HALF = D_HEAD // 2
ROPE_THETA = 10000.0
N_CORES = 8
C = 256  # channels per core (4 heads x 64)
SWAP32 = [i ^ 1 for i in range(32)]


def _body(nc, tc, L, pp, rtp, ptp, nrp, osp, scrp):
    n_qc = L // 512

    xt_d = nc.dram_tensor("xt", [D_MODEL, L], BF16, kind="ExternalInput").ap()
    wq_d = nc.dram_tensor("wqt", [D_MODEL, C], BF16, kind="ExternalInput").ap()
    wk_d = nc.dram_tensor("wkt", [D_MODEL, C], BF16, kind="ExternalInput").ap()
    wv_d = nc.dram_tensor("wvt", [D_MODEL, C], BF16, kind="ExternalInput").ap()
    wo_d = nc.dram_tensor("wot", [C, D_MODEL], BF16, kind="ExternalInput").ap()
    cos_d = nc.dram_tensor("cosb", [128, L], BF16, kind="ExternalInput").ap()
    sin_d = nc.dram_tensor("ssin", [128, L], BF16, kind="ExternalInput").ap()
    mk_d = nc.dram_tensor("masks", [128, 256], BF16, kind="ExternalInput").ap()
    out_d = nc.dram_tensor("out", [L, D_MODEL], BF16, kind="ExternalOutput").ap()

    # ---- persistent SBUF tensors
    wq = pp.tile([128, 8, C], BF16)
    wk = pp.tile([128, 8, C], BF16)
    wv = pp.tile([128, 8, C], BF16)
    wo = pp.tile([128, 2, D_MODEL], BF16)
    cs = pp.tile([128, L], BF16)
    sn = pp.tile([128, L], BF16)
    mks = pp.tile([128, 256], BF16)
    qt_c = [pp.tile([128, 2, 512], BF16, name=f"qt{i}") for i in range(n_qc)]
    kt_c = [pp.tile([128, 2, 512], BF16, name=f"ktc{i}") for i in range(n_qc)]
    # per head h: cols 65h..65h+63 = V, col 65h+64 = ones (denominator row)
    vt_c = [pp.tile([128, 4, 4 * 65], BF16, name=f"vt{i}") for i in range(n_qc)]
    at = pp.tile([128, 2, L], BF16)
    xtc = [[pp.tile([128, 512], BF16, name=f"xt{d}_{q}") for q in range(n_qc)]
           for d in range(8)]

    # ---- loads: priority order (first compute needs wq/wk + x chunk 0)
    nc.sync.dma_start(out=wq[:], in_=wq_d.rearrange("(a p) c -> p a c", p=128))
    nc.sync.dma_start(out=wk[:], in_=wk_d.rearrange("(a p) c -> p a c", p=128))
    for d in range(8):
        nc.sync.dma_start(out=xtc[d][0][:],
                          in_=xt_d[d * 128:(d + 1) * 128, 0:512])
    nc.sync.dma_start(out=cs[:], in_=cos_d)
    nc.sync.dma_start(out=sn[:], in_=sin_d)
    nc.sync.dma_start(out=wv[:], in_=wv_d.rearrange("(a p) c -> p a c", p=128))
    nc.sync.dma_start(out=mks[:], in_=mk_d)
    for q in range(1, n_qc):
        for d in range(8):
            nc.sync.dma_start(out=xtc[d][q][:],
                              in_=xt_d[d * 128:(d + 1) * 128,
                                       q * 512:q * 512 + 512])
    nc.sync.dma_start(out=wo[:], in_=wo_d.rearrange("(a p) e -> p a e", p=128))
    for i in range(n_qc):
        ones_v = vt_c[i][:, :, :].rearrange("p l (h y) -> p l h y", y=65)
        nc.gpsimd.memset(ones_v[:, :, :, 64], 1.0)

    # ---------------- emission units (thunk lists for PE interleaving)
    def qk_units(qc):
        """QK projection + RoPE for chunk qc -> 8 units."""
        units = []
        for nm, w, dstc in (("q", wq, qt_c), ("k", wk, kt_c)):
            for ct in (0, 1):
                st = {}

                def mk_a(nm=nm, w=w, ct=ct, qc=qc, st=st):
                    p = ptp.tile([128, 512], F32, tag="pj",
                                 name=f"pj_{nm}{ct}_{qc}")
                    st["p"] = p
                    for dt_ in range(4):
                        nc.tensor.matmul(
                            p[:], lhsT=w[:, dt_, ct * 128:ct * 128 + 128],
                            rhs=xtc[dt_][qc][:],
                            start=(dt_ == 0), stop=False)

                def mk_b(nm=nm, w=w, ct=ct, qc=qc, st=st, dst=None):
                    p = st["p"]
                    for dt_ in range(4, 8):
                        nc.tensor.matmul(
                            p[:], lhsT=w[:, dt_, ct * 128:ct * 128 + 128],
                            rhs=xtc[dt_][qc][:],
                            start=False, stop=(dt_ == 7))
                    sh = rtp.tile([128, 512], F32, tag="sh",
                                  name=f"sh_{nm}{ct}{qc}")
                    t1 = rtp.tile([128, 512], F32, tag="t1",
                                  name=f"t1_{nm}{ct}{qc}")
                    t2 = rtp.tile([128, 512], F32, tag="t2",
                                  name=f"t2_{nm}{ct}{qc}")
                    ls = qc * 512
                    nc.vector.stream_shuffle(sh[:], p[:], SWAP32)
                    nc.vector.tensor_mul(t1[:], p[:], cs[:, ls:ls + 512])
                    nc.gpsimd.tensor_mul(t2[:], sh[:], sn[:, ls:ls + 512])
                    dstt = (qt_c if nm == "q" else kt_c)[qc]
                    nc.gpsimd.tensor_add(dstt[:, ct, :], t1[:], t2[:])

                units.append(mk_a)
                units.append(mk_b)
        return units

    def v_units(qc):
        """V projection for chunk qc -> 4 units (one per 128-L tile)."""
        units = []
        for j in range(4):
            def mk(qc=qc, j=j):
                pv = ptp.tile([128, 256], F32, tag="pj",
                              name=f"pv_{qc}_{j}")
                lt = qc * 4 + j
                for dt_ in range(8):
                    nc.tensor.matmul(
                        pv[:], lhsT=xtc[dt_][qc][:, j * 128:j * 128 + 128],
                        rhs=wv[:, dt_, :],
                        start=(dt_ == 0), stop=(dt_ == 7))
                ov = vt_c[qc][:, j, :].rearrange("p (h y) -> p h y", y=65)
                iv = pv[:, :].rearrange("p (h y) -> p h y", y=64)
                nc.vector.tensor_copy(ov[:, :, 0:64], iv[:, :, :])
            units.append(mk)
        return units

    def op_units(qc):
        """Output projection for chunk qc -> 8 units."""
        units = []
        for j in range(4):
            for eh in (0, 1):
                def mk(qc=qc, j=j, eh=eh):
                    qtl = qc * 4 + j
                    pout = ptp.tile([128, 512], F32, tag="pj",
                                    name=f"po_{qtl}_{eh}")
                    for ct in (0, 1):
                        nc.tensor.matmul(
                            pout[:],
                            lhsT=at[:, ct, qtl * 128:qtl * 128 + 128],
                            rhs=wo[:, ct, eh * 512:eh * 512 + 512],
                            start=(ct == 0), stop=(ct == 1),
                            skip_group_check=True)
                    stg = osp.tile([128, 512], BF16, tag="stg",
                                   name=f"stg_{qtl}_{eh}")
                    nc.vector.tensor_copy(stg[:], pout[:])
                    nc.sync.dma_start(
                        out=out_d[qtl * 128:qtl * 128 + 128,
                                  eh * 512:eh * 512 + 512],
                        in_=stg[:])
                units.append(mk)
        return units

    # ---------------- attention chunk with interleaved background units
    def attention_chunk(qc, bg):
        qs = qc * 512
        ktmax = qc * 4 + 4
        n_slots = 2 * ktmax
        # pop roughly len(bg)/n_slots units per kt slot
        popped = 0
        done = 0
        for pair in range(2):
            h0, h1 = 2 * pair, 2 * pair + 1
            po = ptp.tile([128, 1024], F32, tag="po", bufs=1,
                          name=f"poacc_{qc}_{pair}")
            for kt in range(ktmax):
                off = kt * 128 - qs
                qlo = max(0, off)
                kc, ko = kt // 4, (kt % 4) * 128
                pt = ptp.tile([128, 1024], F32, tag="sc",
                              name=f"pt_{qc}_{pair}_{kt}")
                for hloc in range(2):
                    nc.tensor.matmul(
                        pt[:, 512 * hloc + qlo:512 * hloc + 512],
                        lhsT=kt_c[kc][64 * hloc:64 * hloc + 64, pair,
                                      ko:ko + 128],
                        rhs=qt_c[qc][64 * hloc:64 * hloc + 64, pair,
                                     qlo:512],
                        start=True, stop=True,
                        tile_position=(64 * hloc, 0),
                        skip_group_check=True)
                ptb = rtp.tile([128, 1024], BF16, tag="ptb", bufs=3,
                               name=f"ptb_{qc}_{pair}_{kt}")
                pv_ps = pt[:, :].rearrange("p (h x) -> p h x", h=2)
                pv_sb = ptb[:, :].rearrange("p (h x) -> p h x", h=2)
                nc.scalar.activation(pv_sb[:, :, qlo:512], pv_ps[:, :, qlo:512],
                                     mybir.ActivationFunctionType.Exp,
                                     scale=1.0 / math.sqrt(D_HEAD))
                if off >= 0:
                    mseg = pv_sb[:, :, qlo:qlo + 128]
                    nc.gpsimd.tensor_mul(
                        mseg, mseg,
                        mks[:, :].rearrange("p (g y) -> p g y", y=128))
                first, last = (kt == 0), (kt == ktmax - 1)
                nc.tensor.matmul(
                    po[0:65, qlo:512],
                    lhsT=vt_c[kc][:, kt % 4, 65 * h0:65 * h0 + 65],
                    rhs=ptb[:, qlo:512],
                    start=first, stop=last, skip_group_check=True)
                nc.tensor.matmul(
                    po[0:65, 512 + qlo:1024],
                    lhsT=vt_c[kc][:, kt % 4, 65 * h1:65 * h1 + 65],
                    rhs=ptb[:, 512 + qlo:1024],
                    start=first, stop=last, skip_group_check=True)
                # interleave background PE work
                done += 1
                want = (len(bg) * done) // n_slots
                while popped < want:
                    bg[popped]()
                    popped += 1
            # ---- finalize pair: denominator row -> DRAM -> partition
            # broadcast -> reciprocal -> normalize on partitions 0:64; h1
            # half shifted up to partitions 64:128 by an SBUF->SBUF DMA.
            atmp = nrp.tile([128, 1024], BF16, tag="atmp",
                            name=f"atmp_{qc}_{pair}")
            nc.vector.tensor_copy(atmp[0:64, :], po[0:64, :])
            rrow = nrp.tile([1, 1024], F32, tag="rrow", name=f"rrow_{qc}_{pair}")
            nc.vector.tensor_copy(rrow[:], po[64:65, :])
            scrt = scrp.tile([1, 1024], F32, tag="scr", name=f"scr_{qc}_{pair}")
            nc.sync.dma_start(out=scrt[:, :], in_=rrow[:])
            pbr = nrp.tile([64, 1024], F32, tag="pbr", name=f"pbr_{qc}_{pair}")
            nc.sync.dma_start(out=pbr[:, :], in_=scrt[:, :].partition_broadcast(64))
            pbi = nrp.tile([64, 1024], F32, tag="pbi", name=f"pbi_{qc}_{pair}")
            nc.vector.reciprocal_approx_fast(out=pbi[:, :], in_=pbr[:, :])
            nc.gpsimd.tensor_mul(at[0:64, pair, qs:qs + 512],
                                 atmp[0:64, 0:512], pbi[:, 0:512])
            th1 = nrp.tile([64, 512], BF16, tag="th1", name=f"th1_{qc}_{pair}")
            nc.gpsimd.tensor_mul(th1[:, :], atmp[0:64, 512:1024],
                                 pbi[:, 512:1024])
            nc.sync.dma_start(out=at[64:128, pair, qs:qs + 512], in_=th1[:, :])
        while popped < len(bg):
            bg[popped]()
            popped += 1

    # ---------------- pipeline
    for u in qk_units(0) + v_units(0):
        u()
    for qc in range(n_qc):
        bg = []
        if qc + 1 < n_qc:
            bg += qk_units(qc + 1) + v_units(qc + 1)
        if qc > 0:
            bg += op_units(qc - 1)
        attention_chunk(qc, bg)
    for u in op_units(n_qc - 1):
        u()


def build_nc(L=2048):
    """Build + compile the per-core Bass program (same NEFF on all 8 cores)."""
    assert L % 512 == 0
    nc = bacc.Bacc("TRN2", target_bir_lowering=False, debug=False,
                   num_devices=N_CORES)
    with tile.TileContext(nc) as tc:
        with tc.tile_pool(name="persist", bufs=1) as pp, \
             tc.tile_pool(name="ropet", bufs=3) as rtp, \
             tc.tile_pool(name="psmix", bufs=2, space="PSUM") as ptp, \
             tc.tile_pool(name="norm", bufs=2) as nrp, \
             tc.tile_pool(name="ostg", bufs=3) as osp, \
             tc.tile_pool(name="riscr", bufs=4, space="DRAM") as scrp:
            _body(nc, tc, L, pp, rtp, ptp, nrp, osp, scrp)
    nc.compile()
    return nc


_NC_CACHE = {}


def _get_nc(L):
    if L not in _NC_CACHE:
        _NC_CACHE[L] = build_nc(L)
    return _NC_CACHE[L]


def make_inputs(x, token_positions, Wq, Wk, Wv, Wo):
    """Host-side shard/layout prep -> list of 8 per-core input dicts."""
    B, L, _ = x.shape
    pos = np.asarray(token_positions).astype(np.float64)
    S = ROPE_THETA ** (-2.0 / D_HEAD)
    thetas = S ** np.arange(HALF, dtype=np.float64)
    ang = pos[:, None] * thetas[None, :]          # [L, 32]
    cosL = np.cos(ang).T                          # [32, L]
    sinL = np.sin(ang).T
    # per-channel tables on the natural (head, dim) layout:
    # row p (within a 64-row head block): pair i = (p%64)//2
    # cosb[p] = cos(theta_i * pos); ssin[p] = -sin if dim even else +sin
    cosb = np.empty((128, L), dtype=np.float64)
    ssin = np.empty((128, L), dtype=np.float64)
    for p in range(128):
        i = (p % 64) // 2
        cosb[p] = cosL[i]
        ssin[p] = -sinL[i] if (p % 2 == 0) else sinL[i]
    cosb = cosb.astype(NPBF16)
    ssin = ssin.astype(NPBF16)

    r = np.arange(128)[:, None]
    col = np.arange(128)[None, :]
    masks = (col >= r).astype(NPBF16)  # [128, 128] tril(keep q>=k)
    masks = np.concatenate([masks, masks], axis=1)  # [128, 256] (both heads)

    xts = [np.ascontiguousarray(x[b].astype(NPBF16).T) for b in range(B)]
    in_maps = []
    shard_cache = {}
    for core in range(N_CORES):
        b, hg = core // 4, core % 4
        if hg not in shard_cache:
            rows = slice(hg * 256, hg * 256 + 256)
            shard_cache[hg] = {
                "wqt": np.ascontiguousarray(Wq[rows].astype(NPBF16).T),
                "wkt": np.ascontiguousarray(Wk[rows].astype(NPBF16).T),
                "wvt": np.ascontiguousarray(Wv[rows].astype(NPBF16).T),
                "wot": np.ascontiguousarray(Wo[:, rows].astype(NPBF16).T),
            }
        m = dict(shard_cache[hg])
        m["xt"] = xts[b]
        m["cosb"] = cosb
        m["ssin"] = ssin
        m["masks"] = masks
        in_maps.append(m)
    return in_maps


def kernel(x, token_positions, Wq, Wk, Wv, Wo):
    x = np.asarray(x); Wq = np.asarray(Wq); Wk = np.asarray(Wk)
    Wv = np.asarray(Wv); Wo = np.asarray(Wo)
    B, L, _ = x.shape
    nc = _get_nc(L)
    in_maps = make_inputs(x, token_positions, Wq, Wk, Wv, Wo)
    res = run_bass_kernel_spmd(nc, in_maps, core_ids=list(range(N_CORES)))
    out = np.zeros((B, L, D_MODEL), dtype=np.float32)
    for core in range(N_CORES):
        out[core // 4] += res.results[core]["out"].astype(np.float32)
    return out
